# revision 16
# baseline (speedup 1.0000x reference)
"""Bass/Trainium2 kernel for nn_Attn_70076686401576 (block-causal-biased MHA).

Math (per reference):
  qkv = x @ Wqkv + bqkv  -> split into q,k,v heads (H=16, hd=64)
  q,k RMS-normalized over head dim (QKNorm, eps=1e-6, scales gq/gk)
  scores = q k^T / sqrt(hd) + M, where M[i,j] = 1.0 for future-frame keys
  attn = softmax(scores); o = attn @ v; out = o @ Wout + bout

Sharding: 16 heads / 8 cores = 2 heads per core (head-parallel).  Each core
computes its 2 heads' q/k/v from the full x (Wqkv column-sharded), runs full
attention for those heads, and produces a partial output via the row-sharded
Wout.  Host sums the 8 partials (+ bout).

v2 design notes (what makes it fast):
  - x^T built by DMA-crossbar transpose (dma_start_transpose, bf16): no PE
    transposes, no PSUM->SBUF copies; frees PE+Vector for real work
  - projection/QKNorm pipeline is tile-granular; element-wise work is split
    across Scalar (cast/square/sqrt), Vector (q-chain) and GpSimd (k-chain)
  - attention: per-ktile PSUM score tile [128, 2head, 512] -> ONE scalar exp
    per ktile ([128,1024]); scalar exp back-to-back is the phase bound
  - v (+ ones column for the softmax denominator) transposed by DMA as well;
    "+1.0 future-frame" mask folded into attn@v with an e-scaled V copy
  - denominator: reciprocal on Vector (f32r), broadcast row 64 -> 64 rows by
    a tiny f32r PE matmul (1 cyc/col, not the 4 cyc/col f32 path)
  - output projection interleaved into the next stripe's ktile loop so the
    exp pipeline never stalls at stripe boundaries; PSUM budget is exactly
    8 banks: scores 2x2 + po 2 + outproj/rb rotation 2
  - outputs DMA'd straight out of PSUM (no staging copy)
"""

import math
import numpy as np

N_TOK_FULL = 4096
D_MODEL = 1024
HD = 64
TPF = 256
EPS = 1e-6
N_CORES = 8


def build_program(n_tok=N_TOK_FULL, debug=False):
    import concourse.bass as bass
    import concourse.tile as tile
    from concourse import bacc, mybir
    from concourse.masks import make_identity
    from contextlib import ExitStack

    f32 = mybir.dt.float32
    f32r = mybir.dt.float32r
    bf16 = mybir.dt.bfloat16
    AF = mybir.ActivationFunctionType
    E_CONST = float(np.exp(1.0))

    D = D_MODEL
    n_ranges = n_tok // 512       # 512-token ranges (projection granularity)
    n_ktiles = n_tok // 128       # 128-token key tiles
    n_stripes = n_tok // 512      # 512-query stripes
    n_frames = n_tok // TPF

    nc = bacc.Bacc("TRN2", target_bir_lowering=False, debug=False,
                   num_devices=N_CORES)
    x_d = nc.dram_tensor("x", [n_tok, D], f32, kind="ExternalInput").ap()
    wqkv_d = nc.dram_tensor("wqkv", [D, 384], f32, kind="ExternalInput").ap()
    bqkv_d = nc.dram_tensor("bqkv", [384], f32, kind="ExternalInput").ap()
    gv_d = nc.dram_tensor("gv", [128, 2], f32, kind="ExternalInput").ap()
    wout_d = nc.dram_tensor("wout", [128, D], f32, kind="ExternalInput").ap()
    out_d = nc.dram_tensor("out", [n_tok, D], f32, kind="ExternalOutput").ap()

    x_t = x_d.rearrange("(t p) d -> t p d", p=128)
    out_t = out_d.rearrange("(t p) d -> t p d", p=128)

    dbg = {}
    if debug:
        for nm, shp in (("dbg_qTb", [128, n_tok]), ("dbg_kTb", [128, n_tok]),
                        ("dbg_va0", [128, n_ktiles * 80]),
                        ("dbg_eva1", [128, n_ktiles * 80]),
                        ("dbg_oTn0", [64, n_tok]), ("dbg_oTn1", [64, n_tok]),
                        ("dbg_et00", [128, 1024]), ("dbg_rb00", [64, 512])):
            dbg[nm] = nc.dram_tensor(nm, shp, mybir.dt.float32,
                                     kind="ExternalOutput").ap()

    with tile.TileContext(nc) as tc:
        ctx = ExitStack()
        sb = ctx.enter_context(tc.tile_pool(name="sb", bufs=1))
        sbp_ctx = ExitStack()
        sbp = sbp_ctx.enter_context(tc.tile_pool(name="sbp", bufs=1))
        ps1_ctx = ExitStack()
        ps1 = ps1_ctx.enter_context(
            tc.tile_pool(name="ps1", bufs=1, space="PSUM"))
        if True:
            # ---- weights/constants (x tile-0 DMA is issued first below) ----
            xinf0 = sbp.tile([128, D], f32, tag="xinf", bufs=3, name="xinf_0")
            nc.sync.dma_start(xinf0[0:64, :], x_t[0][0:64, :])
            nc.sync.dma_start(xinf0[64:128, :], x_t[0][64:128, :])

            wqkvf = sb.tile([128, 8, 384], f32, tag="wqkvf")
            nc.sync.dma_start(wqkvf,
                              wqkv_d.rearrange("(c p) n -> p c n", p=128))
            bq_sb = sb.tile([128, 3], f32, tag="bq")
            nc.sync.dma_start(bq_sb, bqkv_d.rearrange("(c p) -> p c", p=128))
            gv_sb = sb.tile([128, 2], f32, tag="gv")
            nc.sync.dma_start(gv_sb, gv_d)
            wof = sb.tile([128, D], f32, tag="wof")
            nc.sync.dma_start(wof, wout_d)

            blkdf = sb.tile([128, 128], f32, tag="blkdf")
            nc.gpsimd.memset(blkdf, 0.0)
            nc.gpsimd.memset(blkdf[0:64, 0:64], 1.0)
            nc.gpsimd.memset(blkdf[64:128, 64:128], 1.0)
            blkdiag = sb.tile([128, 128], f32r, tag="blkdiag")
            nc.vector.tensor_copy(blkdiag, blkdf)
            onesf = sb.tile([128, 64], f32, tag="onesf")
            nc.gpsimd.memset(onesf, 1.0)
            onesr = sb.tile([128, 64], f32r, tag="onesr")
            nc.vector.tensor_copy(onesr, onesf)
            cb_q = sb.tile([128, 1], f32, tag="cb_q")
            nc.gpsimd.memset(cb_q, 64.0 * EPS)
            cb_k = sb.tile([128, 1], f32, tag="cb_k")
            nc.gpsimd.memset(cb_k, EPS)
            cs_k = sb.tile([128, 1], f32, tag="cs_k")
            nc.gpsimd.memset(cs_k, 1.0 / 64.0)

            wqkv_sb = sb.tile([128, 8, 384], bf16, tag="wqkv")
            nc.vector.tensor_copy(wqkv_sb, wqkvf)
            wo0 = sb.tile([64, D], bf16, tag="wo0")
            nc.scalar.copy(wo0, wof[0:64, :])
            wo1 = sb.tile([64, D], bf16, tag="wo1")
            nc.scalar.copy(wo1, wof[64:128, :])

            # ---- persistent attention operands ----
            qTb = sb.tile([128, n_tok], bf16, tag="qTb")
            kTb = sb.tile([128, n_tok], bf16, tag="kTb")
            oTn0 = sb.tile([64, n_tok], bf16, tag="oTn0")
            oTn1 = sb.tile([64, n_tok], bf16, tag="oTn1")
            # stride 80 (not 65): xbar-transpose writes need 16-element
            # aligned destination offsets
            va0 = sb.tile([128, n_ktiles, 80], bf16, tag="va0")
            va1 = sb.tile([128, n_ktiles, 80], bf16, tag="va1")
            eva0 = sb.tile([128, n_ktiles, 80], bf16, tag="eva0")
            eva1 = sb.tile([128, n_ktiles, 80], bf16, tag="eva1")
            # softmax-denominator ones columns (e-scaled in the eva copies)
            nc.gpsimd.memset(va0[:, :, 64:65], 1.0)
            nc.gpsimd.memset(va1[:, :, 64:65], 1.0)
            nc.gpsimd.memset(eva0[:, :, 64:65], E_CONST)
            nc.gpsimd.memset(eva1[:, :, 64:65], E_CONST)

            # ================= phase 1: projection + QKNorm =================
            for r in range(n_ranges):
                xTr = sbp.tile([128, 8, 512], bf16, tag="xT", bufs=2)
                for tt in range(4):
                    gt = r * 4 + tt
                    if gt == 0:
                        xinf = xinf0
                    else:
                        xinf = sbp.tile([128, D], f32, tag="xinf", bufs=3,
                                        name=f"xinf_{gt}")
                        nc.sync.dma_start(xinf[0:64, :], x_t[gt][0:64, :])
                        nc.sync.dma_start(xinf[64:128, :], x_t[gt][64:128, :])
                    xin = sbp.tile([128, D], bf16, tag="xin", bufs=3,
                                   name=f"xin_{gt}")
                    nc.scalar.copy(xin, xinf)
                    # DMA crossbar transpose: [128 tok, 1024] -> [128, 8, 128]
                    nc.sync.dma_start_transpose(
                        xTr[:, :, tt * 128:(tt + 1) * 128], xin)

                pj = []
                for oc in range(3):
                    pj_oc = ps1.tile([128, 512], f32, tag=f"pj{oc}", bufs=2,
                                     name=f"pj{oc}_{r}")
                    pj.append(pj_oc)
                for dc in range(8):
                    for oc in range(3):
                        nc.tensor.matmul(
                            pj[oc],
                            wqkv_sb[:, dc, oc * 128:(oc + 1) * 128],
                            xTr[:, dc, :],
                            start=(dc == 0), stop=(dc == 7))
                sl = slice(r * 512, (r + 1) * 512)

                # biases: q on vector, k on gpsimd, v on vector (bf16 out)
                qTr = sbp.tile([128, 512], f32r, tag="qTr", bufs=2,
                               name=f"qTr_{r}")
                nc.vector.tensor_scalar_add(qTr, pj[0], bq_sb[:, 0:1])
                kTr = sbp.tile([128, 512], f32r, tag="kTr", bufs=2,
                               name=f"kTr_{r}")
                nc.scalar.activation(kTr, pj[1], AF.Identity,
                                     bias=bq_sb[:, 1:2], scale=1.0)
                vTb = sbp.tile([128, 512], bf16, tag="vTb", bufs=2,
                               name=f"vTb_{r}")
                nc.vector.tensor_scalar_add(vTb, pj[2], bq_sb[:, 2:3])

                # QKNorm: rsqrt(mean(q^2)+eps); 1/sqrt(hd)=0.125 folded into
                # the q branch via sqrt(sumsq + 64*eps).  q-chain muls on
                # Vector, k-chain muls on GpSimd (recip is Vector-only).
                for which, blk, blkb, eng in (
                        ("q", qTr, qTb, nc.vector),
                        ("k", kTr, kTb, nc.gpsimd)):
                    sq = sbp.tile([128, 512], f32r, tag=f"sq{which}", bufs=2,
                                  name=f"sq_{r}_{which}")
                    nc.scalar.activation(sq, blk, AF.Square)
                    ps_r = ps1.tile([128, 512], f32, tag="psr", bufs=2,
                                    name=f"psr_{r}_{which}")
                    nc.tensor.matmul(ps_r, blkdiag, sq, start=True, stop=True)
                    sqs = sbp.tile([128, 512], f32, tag=f"sqs{which}", bufs=2,
                                   name=f"sqs_{r}_{which}")
                    if which == "q":
                        nc.scalar.activation(sqs, ps_r, AF.Sqrt,
                                             bias=cb_q, scale=1.0)
                    else:
                        nc.scalar.activation(sqs, ps_r, AF.Sqrt,
                                             bias=cb_k, scale=cs_k)
                    rs = sbp.tile([128, 512], f32, tag=f"rs{which}", bufs=2,
                                  name=f"rs_{r}_{which}")
                    nc.vector.reciprocal_approx_fast(rs, sqs)
                    gcol = 0 if which == "q" else 1
                    eng.tensor_scalar_mul(rs, rs, gv_sb[:, gcol:gcol + 1])
                    eng.tensor_mul(blkb[:, sl], blk, rs)

                # V -> va/eva for this range's 4 ktiles (DMA transpose + the
                # e-scaled copies for the future-frame mask)
                for i in range(4):
                    kt = 4 * r + i
                    ks = slice(i * 128, (i + 1) * 128)
                    nc.sync.dma_start_transpose(va0[:, kt, 0:64], vTb[0:64, ks])
                    nc.sync.dma_start_transpose(va1[:, kt, 0:64],
                                                vTb[64:128, ks])
                    nc.vector.tensor_scalar_mul(eva0[:, kt, 0:64],
                                                va0[:, kt, 0:64], E_CONST)
                    nc.gpsimd.tensor_scalar_mul(eva1[:, kt, 0:64],
                                                va1[:, kt, 0:64], E_CONST)

            if debug:
                for nm, t in (("dbg_qTb", qTb), ("dbg_kTb", kTb)):
                    stg = sb.tile([128, n_tok], f32, tag=f"stg_{nm}")
                    nc.scalar.copy(stg, t)
                    nc.sync.dma_start(dbg[nm], stg)
                for nm, t in (("dbg_va0", va0), ("dbg_eva1", eva1)):
                    stg = sb.tile([128, n_ktiles * 80], f32, tag=f"stg_{nm}")
                    nc.scalar.copy(stg.rearrange("p (k c) -> p k c", c=80), t)
                    nc.sync.dma_start(dbg[nm], stg)

            # ================= phase 2: attention =================
            sbp_ctx.close()
            ps1_ctx.close()
            ps2_ctx = ExitStack()
            ps2 = ps2_ctx.enter_context(
                tc.tile_pool(name="ps2", bufs=1, space="PSUM"))
            sba_ctx = ExitStack()
            sba = sba_ctx.enter_context(tc.tile_pool(name="sba", bufs=1))

            vab = (va0, va1)
            evab = (eva0, eva1)

            def emit_norm(s, po):
                """Denominator recip + broadcast + normalize for stripe s."""
                qsl = slice(s * 512, (s + 1) * 512)
                rb = []
                for h in range(2):
                    # custom-DVE ops misbehave at base_partition != 0: compute
                    # recip over all 65 rows; only row 64 (denominator) is
                    # consumed by the broadcast matmul.  f32r keeps the
                    # broadcast matmul on the 1 cyc/col path.
                    rd = sba.tile([65, 512], f32, tag="rd", bufs=2,
                                  name=f"rd_{s}_{h}")
                    nc.vector.reciprocal_approx_fast(rd, po[h])
                    rdr = sba.tile([65, 512], f32r, tag="rdr", bufs=2,
                                   name=f"rdr_{s}_{h}")
                    nc.vector.tensor_copy(rdr[64:65, :], rd[64:65, :])
                    ps_rb = ps2.tile([64, 512], f32, tag="pso", bufs=2,
                                     name=f"psrb_{s}_{h}")
                    nc.tensor.matmul(ps_rb, onesr[64:65, :], rdr[64:65, :],
                                     start=True, stop=True,
                                     tile_position=(64, 0))
                    rb_sb = sba.tile([64, 512], f32, tag="rb", bufs=2,
                                     name=f"rb_{s}_{h}")
                    nc.vector.tensor_copy(rb_sb, ps_rb)
                    rb.append(rb_sb)
                    if debug and s == 0 and h == 0:
                        nc.sync.dma_start(dbg["dbg_rb00"], rb_sb)
                nc.vector.tensor_mul(oTn0[:, qsl], po[0][0:64, :], rb[0])
                nc.vector.tensor_mul(oTn1[:, qsl], po[1][0:64, :], rb[1])

            def emit_outproj_tt(s, tt):
                """Output projection + DMA for token-tile tt of stripe s."""
                t0 = s * 512 + tt * 128
                gt = s * 4 + tt
                for half in range(2):
                    nsl = slice(half * 512, (half + 1) * 512)
                    ps_o = ps2.tile([128, 512], f32, tag="pso", bufs=2,
                                    name=f"pso_{s}_{tt}_{half}")
                    nc.tensor.matmul(ps_o, oTn0[:, t0:t0 + 128], wo0[:, nsl],
                                     start=True, stop=False)
                    nc.tensor.matmul(ps_o, oTn1[:, t0:t0 + 128], wo1[:, nsl],
                                     start=False, stop=True)
                    ob = sba.tile([128, 512], f32, tag="ob", bufs=4,
                                  name=f"ob_{s}_{tt}_{half}")
                    nc.vector.tensor_copy(ob, ps_o)
                    nc.sync.dma_start(out_t[gt][:, nsl], ob)

            pending = None
            for s in range(n_stripes):
                qsl = slice(s * 512, (s + 1) * 512)
                po = [ps2.tile([65, 512], f32, tag=f"po{h}", bufs=1,
                               name=f"po{h}_{s}")
                      for h in range(2)]
                for kt in range(n_ktiles):
                    sg = ps2.tile([128, 2, 512], f32, tag="sg", bufs=2,
                                  name=f"sg_{s}_{kt}")
                    for h in range(2):
                        hp = slice(h * 64, (h + 1) * 64)
                        nc.tensor.matmul(
                            sg[:, h, :],
                            kTb[hp, kt * 128:(kt + 1) * 128],
                            qTb[hp, qsl],
                            start=True, stop=True,
                            tile_position=(h * 64, 0))
                    et = sba.tile([128, 2, 512], bf16, tag="et", bufs=6,
                                  name=f"et_{s}_{kt}")
                    nc.scalar.activation(et, sg, AF.Exp)
                    if debug and s == 0 and kt == 0:
                        stg = sba.tile([128, 1024], f32, tag="stg_et")
                        nc.vector.tensor_copy(
                            stg.rearrange("p (h c) -> p h c", c=512), et)
                        nc.sync.dma_start(dbg["dbg_et00"], stg)

                    fk = kt // 2
                    first = (kt == 0)
                    last = (kt == n_ktiles - 1)
                    for h in range(2):
                        rhs = et[:, h, :]
                        if fk == 2 * s + 1:
                            # key frame == 2nd query frame of the stripe:
                            # first 256 queries see it as future (e*V)
                            nc.tensor.matmul(
                                po[h][:, 0:256],
                                evab[h][:, kt, 0:65],
                                rhs[:, 0:256],
                                start=False, stop=False)
                            # stop only on the final matmul (the whole
                            # [65,512] tile is one 2KB psum zero region)
                            nc.tensor.matmul(
                                po[h][:, 256:512],
                                vab[h][:, kt, 0:65],
                                rhs[:, 256:512],
                                start=False, stop=last)
                        else:
                            vv = evab[h] if fk > 2 * s + 1 else vab[h]
                            nc.tensor.matmul(
                                po[h][:, :],
                                vv[:, kt, 0:65],
                                rhs,
                                start=first, stop=last)

                    # previous stripe's normalize + output projection,
                    # spread across this stripe's early ktiles so the exp
                    # pipeline never starves
                    if pending is not None:
                        if kt == 0:
                            emit_norm(*pending)
                        elif 2 <= kt <= 5:
                            emit_outproj_tt(pending[0], kt - 2)
                        if kt == 5:
                            pending = None
                pending = (s, po)
            emit_norm(*pending)
            for tt in range(4):
                emit_outproj_tt(pending[0], tt)

            if debug:
                for nm, t in (("dbg_oTn0", oTn0), ("dbg_oTn1", oTn1)):
                    stg = sba.tile([64, n_tok], f32, tag=f"stg_{nm}")
                    nc.scalar.copy(stg, t)
                    nc.sync.dma_start(dbg[nm], stg)

            sba_ctx.close()
            ps2_ctx.close()
            ctx.close()

    nc.compile()
    return nc


def shard_inputs(x, Wqkv, bqkv, gq, gk, Wout, n_tok):
    """Build the 8 per-core input maps (head-parallel sharding)."""
    D = D_MODEL
    in_maps = []
    for c in range(N_CORES):
        cs = slice(128 * c, 128 * (c + 1))
        wq = Wqkv[:, cs]
        wk = Wqkv[:, D + 128 * c:D + 128 * (c + 1)]
        wv = Wqkv[:, 2 * D + 128 * c:2 * D + 128 * (c + 1)]
        wqkv_s = np.ascontiguousarray(np.concatenate([wq, wk, wv], axis=1),
                                      dtype=np.float32)
        bq = bqkv[cs]
        bk = bqkv[D + 128 * c:D + 128 * (c + 1)]
        bv = bqkv[2 * D + 128 * c:2 * D + 128 * (c + 1)]
        bqkv_s = np.ascontiguousarray(np.concatenate([bq, bk, bv]),
                                      dtype=np.float32)
        gv = np.stack([np.concatenate([gq, gq]),
                       np.concatenate([gk, gk])], axis=1).astype(np.float32)
        wout_s = np.ascontiguousarray(Wout[cs, :], dtype=np.float32)
        in_maps.append({
            "x": np.ascontiguousarray(x[:n_tok], dtype=np.float32),
            "wqkv": wqkv_s,
            "bqkv": bqkv_s,
            "gv": np.ascontiguousarray(gv),
            "wout": wout_s,
        })
    return in_maps


_PROGRAM_CACHE = {}


def _get_program(n_tok):
    if n_tok not in _PROGRAM_CACHE:
        _PROGRAM_CACHE[n_tok] = build_program(n_tok)
    return _PROGRAM_CACHE[n_tok]


def run_sharded(inputs, trace=False, tmpdir=None):
    """Run the SPMD kernel; returns (full_output [1,N,D], BassKernelResults)."""
    from concourse.bass_utils import run_bass_kernel_spmd

    x = np.asarray(inputs["x"], dtype=np.float32)
    Wqkv = np.asarray(inputs["Wqkv"], dtype=np.float32)
    bqkv = np.asarray(inputs["bqkv"], dtype=np.float32)
    Wout = np.asarray(inputs["Wout"], dtype=np.float32)
    bout = np.asarray(inputs["bout"], dtype=np.float32)
    gq = np.asarray(inputs["gq"], dtype=np.float32)
    gk = np.asarray(inputs["gk"], dtype=np.float32)
    tpf = int(np.asarray(inputs["tokens_per_frame"]))
    assert tpf == TPF, f"kernel hardcodes tokens_per_frame={TPF}, got {tpf}"

    B, N, D = x.shape
    assert B == 1 and D == D_MODEL
    x2 = x[0]

    nc = _get_program(N)
    in_maps = shard_inputs(x2, Wqkv, bqkv, gq, gk, Wout, N)
    res = run_bass_kernel_spmd(nc, in_maps, list(range(N_CORES)),
                               trace=trace, tmpdir=tmpdir)
    acc = res.results[0]["out"].astype(np.float32)
    for c in range(1, N_CORES):
        acc = acc + res.results[c]["out"]
    if np.any(bout):
        acc = acc + bout[None, :]
    return acc[None], res


def kernel(**inputs):
    out, _ = run_sharded(inputs)
    return out


# revision 22
# speedup vs baseline: 1.4904x; 1.4904x over previous
"""Bass/Trainium2 kernel for nn_Attn_70076686401576 (block-causal-biased MHA).

Math (per reference):
  qkv = x @ Wqkv + bqkv  -> split into q,k,v heads (H=16, hd=64)
  q,k RMS-normalized over head dim (QKNorm, eps=1e-6, scales gq/gk)
  scores = q k^T / sqrt(hd) + M, where M[i,j] = 1.0 for future-frame keys
  attn = softmax(scores); o = attn @ v; out = o @ Wout + bout

Sharding: 16 heads / 8 cores = 2 heads per core (head-parallel).  Each core
computes its 2 heads' q/k/v from the full x (Wqkv column-sharded), runs full
attention for those heads, and produces a partial output via the row-sharded
Wout.  Host sums the 8 partials (+ bout).

v3 design notes:
  - x loaded ONE DMA per 512-token range ([128,4,1024] f32), cast to bf16 on
    Scalar, transposed by ONE DMA-crossbar transpose per range (the xbar
    transpose blocks its issue queue ~1.2us regardless of size, so batch big)
  - v transposed per-range the same way (2 calls); va tiles strided 80 (the
    xbar needs 16-element-aligned destination offsets)
  - QKNorm chains split across Vector (biases/recip/muls) and Scalar
    (square/sqrt); GpSimd only does memsets (its tensor ops are ~10x slower
    than DVE and it cannot touch PSUM)
  - attention: per-ktile score tile [128, 2head, 512] in PSUM -> ONE scalar
    exp per ktile; scalar exp back-to-back is the phase bound (~285us)
  - softmax denominator: ones-column in V -> po row 64; row copied to SBUF,
    transposed to a [128 tok, tt, h] column layout by a tiny SBUF->SBUF DMA,
    reciprocal'd on Vector, and applied as a per-PARTITION scalar during the
    output-projection PSUM drain (heads kept in separate PSUM tiles) -- no
    PE broadcast matmul, near-zero exp-pipeline stall at stripe boundaries
  - PSUM budget exactly 8 banks: scores 2x2 + po 2 + outproj 2
"""

import math
import numpy as np

N_TOK_FULL = 4096
D_MODEL = 1024
HD = 64
TPF = 256
EPS = 1e-6
N_CORES = 8


def build_program(n_tok=N_TOK_FULL, debug=False):
    import concourse.bass as bass
    import concourse.tile as tile
    from concourse import bacc, mybir
    from contextlib import ExitStack

    f32 = mybir.dt.float32
    f32r = mybir.dt.float32r
    bf16 = mybir.dt.bfloat16
    AF = mybir.ActivationFunctionType
    E_CONST = float(np.exp(1.0))

    D = D_MODEL
    n_ranges = n_tok // 512
    n_ktiles = n_tok // 128
    n_stripes = n_tok // 512

    nc = bacc.Bacc("TRN2", target_bir_lowering=False, debug=False,
                   num_devices=N_CORES)
    x_d = nc.dram_tensor("x", [n_tok, D], f32, kind="ExternalInput").ap()
    wqkv_d = nc.dram_tensor("wqkv", [D, 384], f32, kind="ExternalInput").ap()
    bqkv_d = nc.dram_tensor("bqkv", [384], f32, kind="ExternalInput").ap()
    gv_d = nc.dram_tensor("gv", [128, 2], f32, kind="ExternalInput").ap()
    wout_d = nc.dram_tensor("wout", [128, D], f32, kind="ExternalInput").ap()
    out_d = nc.dram_tensor("out", [n_tok, D], f32, kind="ExternalOutput").ap()
    # DRAM scratch for the denominator transpose (SBUF->SBUF DMAs cannot
    # map a free dim onto partitions; DRAM round-trip can)
    zscr_d = nc.dram_tensor("zscr", [n_tok // 512, 2, 512], f32,
                            kind="Internal").ap()

    x_r = x_d.rearrange("(r t p) d -> r p t d", p=128, t=4)
    out_t = out_d.rearrange("(t p) d -> t p d", p=128)

    dbg = {}
    if debug:
        for nm, shp in (("dbg_qTb", [128, n_tok]), ("dbg_kTb", [128, n_tok]),
                        ("dbg_va0", [128, n_ktiles * 80]),
                        ("dbg_eva1", [128, n_ktiles * 80]),
                        ("dbg_oTn0", [64, n_tok]), ("dbg_oTn1", [64, n_tok]),
                        ("dbg_zr0", [128, 8])):
            dbg[nm] = nc.dram_tensor(nm, shp, mybir.dt.float32,
                                     kind="ExternalOutput").ap()

    with tile.TileContext(nc) as tc:
        ctx = ExitStack()
        sb = ctx.enter_context(tc.tile_pool(name="sb", bufs=1))
        sbp_ctx = ExitStack()
        sbp = sbp_ctx.enter_context(tc.tile_pool(name="sbp", bufs=1))
        ps1_ctx = ExitStack()
        ps1 = ps1_ctx.enter_context(
            tc.tile_pool(name="ps1", bufs=1, space="PSUM"))
        if True:
            # ---- x range-0 DMA first, then weights/constants ----
            xin0 = sbp.tile([128, 4, D], f32, tag="xinf", bufs=2,
                            name="xinf_0")
            nc.sync.dma_start(xin0, x_r[0])

            wqkvf = sb.tile([128, 8, 384], f32, tag="wqkvf")
            nc.sync.dma_start(wqkvf,
                              wqkv_d.rearrange("(c p) n -> p c n", p=128))
            bq_sb = sb.tile([128, 3], f32, tag="bq")
            nc.sync.dma_start(bq_sb, bqkv_d.rearrange("(c p) -> p c", p=128))
            gv_sb = sb.tile([128, 2], f32, tag="gv")
            nc.sync.dma_start(gv_sb, gv_d)
            wof = sb.tile([128, D], f32, tag="wof")
            nc.sync.dma_start(wof, wout_d)

            blkdf = sb.tile([128, 128], f32, tag="blkdf")
            nc.gpsimd.memset(blkdf, 0.0)
            nc.gpsimd.memset(blkdf[0:64, 0:64], 1.0)
            nc.gpsimd.memset(blkdf[64:128, 64:128], 1.0)
            blkdiag = sb.tile([128, 128], f32r, tag="blkdiag")
            nc.vector.tensor_copy(blkdiag, blkdf)
            cb_q = sb.tile([128, 1], f32, tag="cb_q")
            nc.gpsimd.memset(cb_q, 64.0 * EPS)
            cb_k = sb.tile([128, 1], f32, tag="cb_k")
            nc.gpsimd.memset(cb_k, EPS)
            cs_k = sb.tile([128, 1], f32, tag="cs_k")
            nc.gpsimd.memset(cs_k, 1.0 / 64.0)

            wqkv_sb = sb.tile([128, 8, 384], bf16, tag="wqkv")
            nc.vector.tensor_copy(wqkv_sb, wqkvf)
            wo0 = sb.tile([64, D], bf16, tag="wo0")
            nc.scalar.copy(wo0, wof[0:64, :])
            wo1 = sb.tile([64, D], bf16, tag="wo1")
            nc.scalar.copy(wo1, wof[64:128, :])

            # ---- persistent attention operands ----
            qTb = sb.tile([128, n_tok], bf16, tag="qTb")
            kTb = sb.tile([128, n_tok], bf16, tag="kTb")
            oTn0 = sb.tile([64, n_tok], bf16, tag="oTn0")
            oTn1 = sb.tile([64, n_tok], bf16, tag="oTn1")
            # stride 80 (not 65): xbar-transpose writes need 16-element
            # aligned destination offsets
            va0 = sb.tile([128, n_ktiles, 80], bf16, tag="va0")
            va1 = sb.tile([128, n_ktiles, 80], bf16, tag="va1")
            eva0 = sb.tile([128, n_ktiles, 80], bf16, tag="eva0")
            eva1 = sb.tile([128, n_ktiles, 80], bf16, tag="eva1")
            nc.gpsimd.memset(va0[:, :, 64:65], 1.0)
            nc.gpsimd.memset(va1[:, :, 64:65], 1.0)
            nc.gpsimd.memset(eva0[:, :, 64:65], E_CONST)
            nc.gpsimd.memset(eva1[:, :, 64:65], E_CONST)

            # ================= phase 1: projection + QKNorm =================
            for r in range(n_ranges):
                if r == 0:
                    xinf = xin0
                else:
                    xinf = sbp.tile([128, 4, D], f32, tag="xinf", bufs=2,
                                    name=f"xinf_{r}")
                    nc.sync.dma_start(xinf, x_r[r])
                xin = sbp.tile([128, 4, D], bf16, tag="xin", bufs=2,
                               name=f"xin_{r}")
                # split the f32->bf16 cast: scalar does 3 tiles, vector 1
                nc.scalar.copy(xin[:, 0:3, :], xinf[:, 0:3, :])
                nc.vector.tensor_copy(xin[:, 3, :], xinf[:, 3, :])
                # ONE xbar transpose per range:
                # [128 tok, (t d)] -> [128, (t dc), 128 tok]
                xTr = sbp.tile([128, 32, 128], bf16, tag="xT", bufs=2,
                               name=f"xTr_{r}")
                nc.sync.dma_start_transpose(
                    xTr, xin.rearrange("p t d -> p (t d)"))
                xTv = xTr.rearrange("p (t c) m -> p t c m", c=8)

                pj = []
                for oc in range(3):
                    pj_oc = ps1.tile([128, 512], f32, tag=f"pj{oc}", bufs=2,
                                     name=f"pj{oc}_{r}")
                    pj.append(pj_oc)
                for dc in range(8):
                    for oc in range(3):
                        nc.tensor.matmul(
                            pj[oc],
                            wqkv_sb[:, dc, oc * 128:(oc + 1) * 128],
                            xTv[:, :, dc, :],
                            start=(dc == 0), stop=(dc == 7))
                sl = slice(r * 512, (r + 1) * 512)

                qTr = sbp.tile([128, 512], f32r, tag="qTr", bufs=2,
                               name=f"qTr_{r}")
                nc.vector.tensor_scalar_add(qTr, pj[0], bq_sb[:, 0:1])
                kTr = sbp.tile([128, 512], f32r, tag="kTr", bufs=2,
                               name=f"kTr_{r}")
                nc.vector.tensor_scalar_add(kTr, pj[1], bq_sb[:, 1:2])
                vTb = sbp.tile([128, 512], bf16, tag="vTb", bufs=2,
                               name=f"vTb_{r}")
                nc.vector.tensor_scalar_add(vTb, pj[2], bq_sb[:, 2:3])

                # QKNorm: rsqrt(mean(q^2)+eps); 1/sqrt(hd)=0.125 folded into
                # the q branch via sqrt(sumsq + 64*eps)
                for which, blk, blkb in (("q", qTr, qTb), ("k", kTr, kTb)):
                    sq = sbp.tile([128, 512], f32r, tag=f"sq{which}", bufs=2,
                                  name=f"sq_{r}_{which}")
                    nc.scalar.activation(sq, blk, AF.Square)
                    ps_r = ps1.tile([128, 512], f32, tag="psr", bufs=2,
                                    name=f"psr_{r}_{which}")
                    nc.tensor.matmul(ps_r, blkdiag, sq, start=True, stop=True)
                    sqs = sbp.tile([128, 512], f32, tag=f"sqs{which}", bufs=2,
                                   name=f"sqs_{r}_{which}")
                    if which == "q":
                        nc.scalar.activation(sqs, ps_r, AF.Sqrt,
                                             bias=cb_q, scale=1.0)
                    else:
                        nc.scalar.activation(sqs, ps_r, AF.Sqrt,
                                             bias=cb_k, scale=cs_k)
                    rs = sbp.tile([128, 512], f32, tag=f"rs{which}", bufs=2,
                                  name=f"rs_{r}_{which}")
                    nc.vector.reciprocal_approx_fast(rs, sqs)
                    gcol = 0 if which == "q" else 1
                    nc.vector.tensor_scalar_mul(rs, rs,
                                                gv_sb[:, gcol:gcol + 1])
                    nc.vector.tensor_mul(blkb[:, sl], blk, rs)

                # V -> va/eva: one xbar transpose per head-half per range
                kts = slice(4 * r, 4 * r + 4)
                nc.sync.dma_start_transpose(va0[:, kts, 0:64], vTb[0:64, :])
                nc.sync.dma_start_transpose(va1[:, kts, 0:64], vTb[64:128, :])
                nc.vector.tensor_scalar_mul(eva0[:, kts, 0:64],
                                            va0[:, kts, 0:64], E_CONST)
                nc.vector.tensor_scalar_mul(eva1[:, kts, 0:64],
                                            va1[:, kts, 0:64], E_CONST)

            if debug:
                for nm, t in (("dbg_qTb", qTb), ("dbg_kTb", kTb)):
                    stg = sb.tile([128, n_tok], f32, tag=f"stg_{nm}")
                    nc.scalar.copy(stg, t)
                    nc.sync.dma_start(dbg[nm], stg)
                for nm, t in (("dbg_va0", va0), ("dbg_eva1", eva1)):
                    stg = sb.tile([128, n_ktiles * 80], f32, tag=f"stg_{nm}")
                    nc.scalar.copy(
                        stg.rearrange("p (k c) -> p k c", c=80), t)
                    nc.sync.dma_start(dbg[nm], stg)

            # ================= phase 2: attention =================
            sbp_ctx.close()
            ps1_ctx.close()
            ps2_ctx = ExitStack()
            ps2 = ps2_ctx.enter_context(
                tc.tile_pool(name="ps2", bufs=1, space="PSUM"))
            sba_ctx = ExitStack()
            sba = sba_ctx.enter_context(tc.tile_pool(name="sba", bufs=1))

            vab = (va0, va1)
            evab = (eva0, eva1)

            def emit_norm(s, po):
                """Free po: copy unnormalized o to SBUF + extract denoms.

                The denominator row (64) of each head's po is copied into
                zrow ([65, 2, 512]: head on the middle dim), then a tiny
                SBUF->SBUF DMA transposes both rows into zcol
                [128 tok, tt, h] and Vector reciprocals it.  The division
                happens later, during the outproj PSUM drain, as a
                per-partition (=per-token) scalar."""
                qsl = slice(s * 512, (s + 1) * 512)
                nc.vector.tensor_copy(oTn0[:, qsl], po[0][0:64, :])
                nc.vector.tensor_copy(oTn1[:, qsl], po[1][0:64, :])
                zrow = sba.tile([65, 2, 512], f32, tag="zrow", bufs=2,
                                name=f"zrow_{s}")
                nc.vector.tensor_copy(zrow[64:65, 0, :], po[0][64:65, :])
                nc.vector.tensor_copy(zrow[64:65, 1, :], po[1][64:65, :])
                # transpose [2, 512] -> [128 tok, 2, 4] via DRAM round-trip
                nc.sync.dma_start(zscr_d[s], zrow[64:65, :, :])
                zcol = sba.tile([128, 2, 4], f32, tag="zcol", bufs=2,
                                name=f"zcol_{s}")
                nc.sync.dma_start(
                    zcol,
                    zscr_d[s].rearrange("h (t p) -> p h t", p=128))
                zr = sba.tile([128, 2, 4], f32, tag="zr", bufs=2,
                              name=f"zr_{s}")
                nc.vector.reciprocal_approx_fast(zr, zcol)
                if debug and s == 0:
                    nc.sync.dma_start(
                        dbg["dbg_zr0"], zr.rearrange("p a b -> p (a b)"))
                return zr

            def emit_outproj(s, zr, tt, half):
                """Output projection for (token-tile, dmodel-half); the two
                heads go to separate PSUM tiles and are combined with the
                per-token 1/Z scalars during the drain."""
                t0 = s * 512 + tt * 128
                gt = s * 4 + tt
                nsl = slice(half * 512, (half + 1) * 512)
                ps_a = ps2.tile([128, 512], f32, tag="pso", bufs=2,
                                name=f"psa_{s}_{tt}_{half}")
                nc.tensor.matmul(ps_a, oTn0[:, t0:t0 + 128], wo0[:, nsl],
                                 start=True, stop=True)
                ps_b = ps2.tile([128, 512], f32, tag="pso", bufs=2,
                                name=f"psb_{s}_{tt}_{half}")
                nc.tensor.matmul(ps_b, oTn1[:, t0:t0 + 128], wo1[:, nsl],
                                 start=True, stop=True)
                tmp = sba.tile([128, 512], f32, tag="obt", bufs=2,
                               name=f"obt_{s}_{tt}_{half}")
                nc.vector.tensor_scalar_mul(tmp, ps_b, zr[:, 1, tt:tt + 1])
                ob = sba.tile([128, 512], f32, tag="ob", bufs=4,
                              name=f"ob_{s}_{tt}_{half}")
                nc.vector.scalar_tensor_tensor(
                    ob, ps_a, zr[:, 0, tt:tt + 1], tmp,
                    op0=mybir.AluOpType.mult, op1=mybir.AluOpType.add)
                nc.sync.dma_start(out_t[gt][:, nsl], ob)

            pending = None
            for s in range(n_stripes):
                qsl = slice(s * 512, (s + 1) * 512)
                po = [ps2.tile([65, 512], f32, tag=f"po{h}", bufs=1,
                               name=f"po{h}_{s}")
                      for h in range(2)]
                for kt in range(n_ktiles):
                    sg = ps2.tile([128, 2, 512], f32, tag="sg", bufs=2,
                                  name=f"sg_{s}_{kt}")
                    for h in range(2):
                        hp = slice(h * 64, (h + 1) * 64)
                        nc.tensor.matmul(
                            sg[:, h, :],
                            kTb[hp, kt * 128:(kt + 1) * 128],
                            qTb[hp, qsl],
                            start=True, stop=True,
                            tile_position=(h * 64, 0))
                    et = sba.tile([128, 2, 512], bf16, tag="et", bufs=6,
                                  name=f"et_{s}_{kt}")
                    nc.scalar.activation(et, sg, AF.Exp)

                    fk = kt // 2
                    first = (kt == 0)
                    last = (kt == n_ktiles - 1)
                    for h in range(2):
                        rhs = et[:, h, :]
                        if fk == 2 * s + 1:
                            # key frame == 2nd query frame of the stripe:
                            # first 256 queries see it as future (e*V)
                            nc.tensor.matmul(
                                po[h][:, 0:256],
                                evab[h][:, kt, 0:65],
                                rhs[:, 0:256],
                                start=False, stop=False)
                            # stop only on the final matmul (the whole
                            # [65,512] tile is one 2KB psum zero region)
                            nc.tensor.matmul(
                                po[h][:, 256:512],
                                vab[h][:, kt, 0:65],
                                rhs[:, 256:512],
                                start=False, stop=last)
                        else:
                            vv = evab[h] if fk > 2 * s + 1 else vab[h]
                            nc.tensor.matmul(
                                po[h][:, :],
                                vv[:, kt, 0:65],
                                rhs,
                                start=first, stop=last)

                    # previous stripe's outproj, spread one (tt, half) per
                    # ktile so the exp pipeline never starves
                    if pending is not None and 4 <= kt < 12:
                        emit_outproj(pending[0], pending[1],
                                     (kt - 4) // 2, (kt - 4) % 2)
                        if kt == 11:
                            pending = None
                # free po quickly: unnormalized o + denominators out
                zr_s = emit_norm(s, po)
                pending = (s, zr_s)
            for tt in range(4):
                for half in range(2):
                    emit_outproj(pending[0], pending[1], tt, half)

            if debug:
                for nm, t in (("dbg_oTn0", oTn0), ("dbg_oTn1", oTn1)):
                    stg = sba.tile([64, n_tok], f32, tag=f"stg_{nm}")
                    nc.scalar.copy(stg, t)
                    nc.sync.dma_start(dbg[nm], stg)

            sba_ctx.close()
            ps2_ctx.close()
            ctx.close()

    nc.compile()
    return nc


def shard_inputs(x, Wqkv, bqkv, gq, gk, Wout, n_tok):
    """Build the 8 per-core input maps (head-parallel sharding)."""
    D = D_MODEL
    in_maps = []
    for c in range(N_CORES):
        cs = slice(128 * c, 128 * (c + 1))
        wq = Wqkv[:, cs]
        wk = Wqkv[:, D + 128 * c:D + 128 * (c + 1)]
        wv = Wqkv[:, 2 * D + 128 * c:2 * D + 128 * (c + 1)]
        wqkv_s = np.ascontiguousarray(np.concatenate([wq, wk, wv], axis=1),
                                      dtype=np.float32)
        bq = bqkv[cs]
        bk = bqkv[D + 128 * c:D + 128 * (c + 1)]
        bv = bqkv[2 * D + 128 * c:2 * D + 128 * (c + 1)]
        bqkv_s = np.ascontiguousarray(np.concatenate([bq, bk, bv]),
                                      dtype=np.float32)
        gv = np.stack([np.concatenate([gq, gq]),
                       np.concatenate([gk, gk])], axis=1).astype(np.float32)
        wout_s = np.ascontiguousarray(Wout[cs, :], dtype=np.float32)
        in_maps.append({
            "x": np.ascontiguousarray(x[:n_tok], dtype=np.float32),
            "wqkv": wqkv_s,
            "bqkv": bqkv_s,
            "gv": np.ascontiguousarray(gv),
            "wout": wout_s,
        })
    return in_maps


_PROGRAM_CACHE = {}


def _get_program(n_tok):
    if n_tok not in _PROGRAM_CACHE:
        _PROGRAM_CACHE[n_tok] = build_program(n_tok)
    return _PROGRAM_CACHE[n_tok]


def run_sharded(inputs, trace=False, tmpdir=None):
    """Run the SPMD kernel; returns (full_output [1,N,D], BassKernelResults)."""
    from concourse.bass_utils import run_bass_kernel_spmd

    x = np.asarray(inputs["x"], dtype=np.float32)
    Wqkv = np.asarray(inputs["Wqkv"], dtype=np.float32)
    bqkv = np.asarray(inputs["bqkv"], dtype=np.float32)
    Wout = np.asarray(inputs["Wout"], dtype=np.float32)
    bout = np.asarray(inputs["bout"], dtype=np.float32)
    gq = np.asarray(inputs["gq"], dtype=np.float32)
    gk = np.asarray(inputs["gk"], dtype=np.float32)
    tpf = int(np.asarray(inputs["tokens_per_frame"]))
    assert tpf == TPF, f"kernel hardcodes tokens_per_frame={TPF}, got {tpf}"

    B, N, D = x.shape
    assert B == 1 and D == D_MODEL
    x2 = x[0]

    nc = _get_program(N)
    in_maps = shard_inputs(x2, Wqkv, bqkv, gq, gk, Wout, N)
    res = run_bass_kernel_spmd(nc, in_maps, list(range(N_CORES)),
                               trace=trace, tmpdir=tmpdir)
    acc = res.results[0]["out"].astype(np.float32)
    for c in range(1, N_CORES):
        acc = acc + res.results[c]["out"]
    if np.any(bout):
        acc = acc + bout[None, :]
    return acc[None], res


def kernel(**inputs):
    out, _ = run_sharded(inputs)
    return out


# revision 23
# speedup vs baseline: 1.4935x; 1.0021x over previous
"""Bass/Trainium2 kernel for nn_Attn_70076686401576 (block-causal-biased MHA).

Math (per reference):
  qkv = x @ Wqkv + bqkv  -> split into q,k,v heads (H=16, hd=64)
  q,k RMS-normalized over head dim (QKNorm, eps=1e-6, scales gq/gk)
  scores = q k^T / sqrt(hd) + M, where M[i,j] = 1.0 for future-frame keys
  attn = softmax(scores); o = attn @ v; out = o @ Wout + bout

Sharding: 16 heads / 8 cores = 2 heads per core (head-parallel).  Each core
computes its 2 heads' q/k/v from the full x (Wqkv column-sharded), runs full
attention for those heads, and produces a partial output via the row-sharded
Wout.  Host sums the 8 partials (+ bout).

v3 design notes:
  - x loaded ONE DMA per 512-token range ([128,4,1024] f32), cast to bf16 on
    Scalar, transposed by ONE DMA-crossbar transpose per range (the xbar
    transpose blocks its issue queue ~1.2us regardless of size, so batch big)
  - v transposed per-range the same way (2 calls); va tiles strided 80 (the
    xbar needs 16-element-aligned destination offsets)
  - QKNorm chains split across Vector (biases/recip/muls) and Scalar
    (square/sqrt); GpSimd only does memsets (its tensor ops are ~10x slower
    than DVE and it cannot touch PSUM)
  - attention: per-ktile score tile [128, 2head, 512] in PSUM -> ONE scalar
    exp per ktile; scalar exp back-to-back is the phase bound (~285us)
  - softmax denominator: ones-column in V -> po row 64; row copied to SBUF,
    transposed to a [128 tok, tt, h] column layout by a tiny SBUF->SBUF DMA,
    reciprocal'd on Vector, and applied as a per-PARTITION scalar during the
    output-projection PSUM drain (heads kept in separate PSUM tiles) -- no
    PE broadcast matmul, near-zero exp-pipeline stall at stripe boundaries
  - PSUM budget exactly 8 banks: scores 2x2 + po 2 + outproj 2
"""

import math
import numpy as np

N_TOK_FULL = 4096
D_MODEL = 1024
HD = 64
TPF = 256
EPS = 1e-6
N_CORES = 8


def build_program(n_tok=N_TOK_FULL, debug=False):
    import concourse.bass as bass
    import concourse.tile as tile
    from concourse import bacc, mybir
    from contextlib import ExitStack

    f32 = mybir.dt.float32
    f32r = mybir.dt.float32r
    bf16 = mybir.dt.bfloat16
    AF = mybir.ActivationFunctionType
    E_CONST = float(np.exp(1.0))

    D = D_MODEL
    n_ranges = n_tok // 512
    n_ktiles = n_tok // 128
    n_stripes = n_tok // 512

    nc = bacc.Bacc("TRN2", target_bir_lowering=False, debug=False,
                   num_devices=N_CORES)
    x_d = nc.dram_tensor("x", [n_tok, D], f32, kind="ExternalInput").ap()
    wqkv_d = nc.dram_tensor("wqkv", [D, 384], f32, kind="ExternalInput").ap()
    bqkv_d = nc.dram_tensor("bqkv", [384], f32, kind="ExternalInput").ap()
    gv_d = nc.dram_tensor("gv", [128, 2], f32, kind="ExternalInput").ap()
    wout_d = nc.dram_tensor("wout", [128, D], f32, kind="ExternalInput").ap()
    out_d = nc.dram_tensor("out", [n_tok, D], f32, kind="ExternalOutput").ap()
    # DRAM scratch for the denominator transpose (SBUF->SBUF DMAs cannot
    # map a free dim onto partitions; DRAM round-trip can)
    zscr_d = nc.dram_tensor("zscr", [n_tok // 512, 2, 512], f32,
                            kind="Internal").ap()

    x_r = x_d.rearrange("(r t p) d -> r p t d", p=128, t=4)
    out_t = out_d.rearrange("(t p) d -> t p d", p=128)

    dbg = {}
    if debug:
        for nm, shp in (("dbg_qTb", [128, n_tok]), ("dbg_kTb", [128, n_tok]),
                        ("dbg_va0", [128, n_ktiles * 80]),
                        ("dbg_eva1", [128, n_ktiles * 80]),
                        ("dbg_oTn0", [64, n_tok]), ("dbg_oTn1", [64, n_tok]),
                        ("dbg_zr0", [128, 8])):
            dbg[nm] = nc.dram_tensor(nm, shp, mybir.dt.float32,
                                     kind="ExternalOutput").ap()

    with tile.TileContext(nc) as tc:
        ctx = ExitStack()
        sb = ctx.enter_context(tc.tile_pool(name="sb", bufs=1))
        sbp_ctx = ExitStack()
        sbp = sbp_ctx.enter_context(tc.tile_pool(name="sbp", bufs=1))
        ps1_ctx = ExitStack()
        ps1 = ps1_ctx.enter_context(
            tc.tile_pool(name="ps1", bufs=1, space="PSUM"))
        if True:
            # ---- x range-0 DMA first, then weights/constants ----
            xin0 = sbp.tile([128, 4, D], f32, tag="xinf", bufs=3,
                            name="xinf_0")
            nc.sync.dma_start(xin0, x_r[0])

            wqkvf = sb.tile([128, 8, 384], f32, tag="wqkvf")
            nc.sync.dma_start(wqkvf,
                              wqkv_d.rearrange("(c p) n -> p c n", p=128))
            bq_sb = sb.tile([128, 3], f32, tag="bq")
            nc.sync.dma_start(bq_sb, bqkv_d.rearrange("(c p) -> p c", p=128))
            gv_sb = sb.tile([128, 2], f32, tag="gv")
            nc.sync.dma_start(gv_sb, gv_d)
            wof = sb.tile([128, D], f32, tag="wof")
            nc.sync.dma_start(wof, wout_d)

            blkdf = sb.tile([128, 128], f32, tag="blkdf")
            nc.gpsimd.memset(blkdf, 0.0)
            nc.gpsimd.memset(blkdf[0:64, 0:64], 1.0)
            nc.gpsimd.memset(blkdf[64:128, 64:128], 1.0)
            blkdiag = sb.tile([128, 128], f32r, tag="blkdiag")
            nc.vector.tensor_copy(blkdiag, blkdf)
            cb_q = sb.tile([128, 1], f32, tag="cb_q")
            nc.gpsimd.memset(cb_q, 64.0 * EPS)
            cb_k = sb.tile([128, 1], f32, tag="cb_k")
            nc.gpsimd.memset(cb_k, EPS)
            cs_k = sb.tile([128, 1], f32, tag="cs_k")
            nc.gpsimd.memset(cs_k, 1.0 / 64.0)

            wqkv_sb = sb.tile([128, 8, 384], bf16, tag="wqkv")
            nc.vector.tensor_copy(wqkv_sb, wqkvf)
            wo0 = sb.tile([64, D], bf16, tag="wo0")
            nc.scalar.copy(wo0, wof[0:64, :])
            wo1 = sb.tile([64, D], bf16, tag="wo1")
            nc.scalar.copy(wo1, wof[64:128, :])

            # ---- persistent attention operands ----
            qTb = sb.tile([128, n_tok], bf16, tag="qTb")
            kTb = sb.tile([128, n_tok], bf16, tag="kTb")
            # stride 80 (not 65): xbar-transpose writes need 16-element
            # aligned destination offsets
            va0 = sb.tile([128, n_ktiles, 80], bf16, tag="va0")
            va1 = sb.tile([128, n_ktiles, 80], bf16, tag="va1")
            eva0 = sb.tile([128, n_ktiles, 80], bf16, tag="eva0")
            eva1 = sb.tile([128, n_ktiles, 80], bf16, tag="eva1")
            nc.gpsimd.memset(va0[:, :, 64:65], 1.0)
            nc.gpsimd.memset(va1[:, :, 64:65], 1.0)
            nc.gpsimd.memset(eva0[:, :, 64:65], E_CONST)
            nc.gpsimd.memset(eva1[:, :, 64:65], E_CONST)

            # ================= phase 1: projection + QKNorm =================
            for r in range(n_ranges):
                if r == 0:
                    xinf = xin0
                else:
                    xinf = sbp.tile([128, 4, D], f32, tag="xinf", bufs=3,
                                    name=f"xinf_{r}")
                    nc.sync.dma_start(xinf, x_r[r])
                xin = sbp.tile([128, 4, D], bf16, tag="xin", bufs=3,
                               name=f"xin_{r}")
                # split the f32->bf16 cast: scalar does 3 tiles, vector 1
                nc.scalar.copy(xin[:, 0:3, :], xinf[:, 0:3, :])
                nc.vector.tensor_copy(xin[:, 3, :], xinf[:, 3, :])
                # ONE xbar transpose per range:
                # [128 tok, (t d)] -> [128, (t dc), 128 tok]
                xTr = sbp.tile([128, 32, 128], bf16, tag="xT", bufs=3,
                               name=f"xTr_{r}")
                nc.sync.dma_start_transpose(
                    xTr, xin.rearrange("p t d -> p (t d)"))
                xTv = xTr.rearrange("p (t c) m -> p t c m", c=8)

                pj = []
                for oc in range(3):
                    pj_oc = ps1.tile([128, 512], f32, tag=f"pj{oc}", bufs=2,
                                     name=f"pj{oc}_{r}")
                    pj.append(pj_oc)
                for dc in range(8):
                    for oc in range(3):
                        nc.tensor.matmul(
                            pj[oc],
                            wqkv_sb[:, dc, oc * 128:(oc + 1) * 128],
                            xTv[:, :, dc, :],
                            start=(dc == 0), stop=(dc == 7))
                sl = slice(r * 512, (r + 1) * 512)

                qTr = sbp.tile([128, 512], f32r, tag="qTr", bufs=2,
                               name=f"qTr_{r}")
                nc.vector.tensor_scalar_add(qTr, pj[0], bq_sb[:, 0:1])
                kTr = sbp.tile([128, 512], f32r, tag="kTr", bufs=2,
                               name=f"kTr_{r}")
                nc.vector.tensor_scalar_add(kTr, pj[1], bq_sb[:, 1:2])
                vTb = sbp.tile([128, 512], bf16, tag="vTb", bufs=2,
                               name=f"vTb_{r}")
                nc.vector.tensor_scalar_add(vTb, pj[2], bq_sb[:, 2:3])

                # QKNorm: rsqrt(mean(q^2)+eps); 1/sqrt(hd)=0.125 folded into
                # the q branch via sqrt(sumsq + 64*eps)
                for which, blk, blkb in (("q", qTr, qTb), ("k", kTr, kTb)):
                    sq = sbp.tile([128, 512], f32r, tag=f"sq{which}", bufs=2,
                                  name=f"sq_{r}_{which}")
                    nc.scalar.activation(sq, blk, AF.Square)
                    ps_r = ps1.tile([128, 512], f32, tag="psr", bufs=2,
                                    name=f"psr_{r}_{which}")
                    nc.tensor.matmul(ps_r, blkdiag, sq, start=True, stop=True)
                    sqs = sbp.tile([128, 512], f32, tag=f"sqs{which}", bufs=2,
                                   name=f"sqs_{r}_{which}")
                    if which == "q":
                        nc.scalar.activation(sqs, ps_r, AF.Sqrt,
                                             bias=cb_q, scale=1.0)
                    else:
                        nc.scalar.activation(sqs, ps_r, AF.Sqrt,
                                             bias=cb_k, scale=cs_k)
                    rs = sbp.tile([128, 512], f32, tag=f"rs{which}", bufs=2,
                                  name=f"rs_{r}_{which}")
                    nc.vector.reciprocal_approx_fast(rs, sqs)
                    gcol = 0 if which == "q" else 1
                    nc.vector.tensor_scalar_mul(rs, rs,
                                                gv_sb[:, gcol:gcol + 1])
                    nc.vector.tensor_mul(blkb[:, sl], blk, rs)

                # V -> va/eva: one xbar transpose per head-half per range
                kts = slice(4 * r, 4 * r + 4)
                nc.sync.dma_start_transpose(va0[:, kts, 0:64], vTb[0:64, :])
                nc.sync.dma_start_transpose(va1[:, kts, 0:64], vTb[64:128, :])
                nc.vector.tensor_scalar_mul(eva0[:, kts, 0:64],
                                            va0[:, kts, 0:64], E_CONST)
                nc.vector.tensor_scalar_mul(eva1[:, kts, 0:64],
                                            va1[:, kts, 0:64], E_CONST)

            if debug:
                for nm, t in (("dbg_qTb", qTb), ("dbg_kTb", kTb)):
                    stg = sb.tile([128, n_tok], f32, tag=f"stg_{nm}")
                    nc.scalar.copy(stg, t)
                    nc.sync.dma_start(dbg[nm], stg)
                for nm, t in (("dbg_va0", va0), ("dbg_eva1", eva1)):
                    stg = sb.tile([128, n_ktiles * 80], f32, tag=f"stg_{nm}")
                    nc.scalar.copy(
                        stg.rearrange("p (k c) -> p k c", c=80), t)
                    nc.sync.dma_start(dbg[nm], stg)

            # ================= phase 2: attention =================
            sbp_ctx.close()
            ps1_ctx.close()
            ps2_ctx = ExitStack()
            ps2 = ps2_ctx.enter_context(
                tc.tile_pool(name="ps2", bufs=1, space="PSUM"))
            sba_ctx = ExitStack()
            sba = sba_ctx.enter_context(tc.tile_pool(name="sba", bufs=1))

            oTn0 = sba.tile([64, n_tok], bf16, tag="oTn0")
            oTn1 = sba.tile([64, n_tok], bf16, tag="oTn1")

            vab = (va0, va1)
            evab = (eva0, eva1)

            def emit_norm(s, po):
                """Free po: copy unnormalized o to SBUF + extract denoms.

                The denominator row (64) of each head's po is copied into
                zrow ([65, 2, 512]: head on the middle dim), then a tiny
                SBUF->SBUF DMA transposes both rows into zcol
                [128 tok, tt, h] and Vector reciprocals it.  The division
                happens later, during the outproj PSUM drain, as a
                per-partition (=per-token) scalar."""
                qsl = slice(s * 512, (s + 1) * 512)
                nc.vector.tensor_copy(oTn0[:, qsl], po[0][0:64, :])
                nc.vector.tensor_copy(oTn1[:, qsl], po[1][0:64, :])
                zrow = sba.tile([65, 2, 512], f32, tag="zrow", bufs=2,
                                name=f"zrow_{s}")
                nc.vector.tensor_copy(zrow[64:65, 0, :], po[0][64:65, :])
                nc.vector.tensor_copy(zrow[64:65, 1, :], po[1][64:65, :])
                # transpose [2, 512] -> [128 tok, 2, 4] via DRAM round-trip
                nc.sync.dma_start(zscr_d[s], zrow[64:65, :, :])
                zcol = sba.tile([128, 2, 4], f32, tag="zcol", bufs=2,
                                name=f"zcol_{s}")
                nc.sync.dma_start(
                    zcol,
                    zscr_d[s].rearrange("h (t p) -> p h t", p=128))
                zr = sba.tile([128, 2, 4], f32, tag="zr", bufs=2,
                              name=f"zr_{s}")
                nc.vector.reciprocal_approx_fast(zr, zcol)
                if debug and s == 0:
                    nc.sync.dma_start(
                        dbg["dbg_zr0"], zr.rearrange("p a b -> p (a b)"))
                return zr

            def emit_outproj(s, zr, tt, half):
                """Output projection for (token-tile, dmodel-half); the two
                heads go to separate PSUM tiles and are combined with the
                per-token 1/Z scalars during the drain."""
                t0 = s * 512 + tt * 128
                gt = s * 4 + tt
                nsl = slice(half * 512, (half + 1) * 512)
                ps_a = ps2.tile([128, 512], f32, tag="pso", bufs=2,
                                name=f"psa_{s}_{tt}_{half}")
                nc.tensor.matmul(ps_a, oTn0[:, t0:t0 + 128], wo0[:, nsl],
                                 start=True, stop=True)
                ps_b = ps2.tile([128, 512], f32, tag="pso", bufs=2,
                                name=f"psb_{s}_{tt}_{half}")
                nc.tensor.matmul(ps_b, oTn1[:, t0:t0 + 128], wo1[:, nsl],
                                 start=True, stop=True)
                tmp = sba.tile([128, 512], f32, tag="obt", bufs=2,
                               name=f"obt_{s}_{tt}_{half}")
                nc.vector.tensor_scalar_mul(tmp, ps_b, zr[:, 1, tt:tt + 1])
                ob = sba.tile([128, 512], f32, tag="ob", bufs=4,
                              name=f"ob_{s}_{tt}_{half}")
                nc.vector.scalar_tensor_tensor(
                    ob, ps_a, zr[:, 0, tt:tt + 1], tmp,
                    op0=mybir.AluOpType.mult, op1=mybir.AluOpType.add)
                nc.sync.dma_start(out_t[gt][:, nsl], ob)

            pending = None
            for s in range(n_stripes):
                qsl = slice(s * 512, (s + 1) * 512)
                po = [ps2.tile([65, 512], f32, tag=f"po{h}", bufs=1,
                               name=f"po{h}_{s}")
                      for h in range(2)]
                for kt in range(n_ktiles):
                    sg = ps2.tile([128, 2, 512], f32, tag="sg", bufs=2,
                                  name=f"sg_{s}_{kt}")
                    for h in range(2):
                        hp = slice(h * 64, (h + 1) * 64)
                        nc.tensor.matmul(
                            sg[:, h, :],
                            kTb[hp, kt * 128:(kt + 1) * 128],
                            qTb[hp, qsl],
                            start=True, stop=True,
                            tile_position=(h * 64, 0))
                    et = sba.tile([128, 2, 512], bf16, tag="et", bufs=6,
                                  name=f"et_{s}_{kt}")
                    nc.scalar.activation(et, sg, AF.Exp)

                    fk = kt // 2
                    first = (kt == 0)
                    last = (kt == n_ktiles - 1)
                    for h in range(2):
                        rhs = et[:, h, :]
                        if fk == 2 * s + 1:
                            # key frame == 2nd query frame of the stripe:
                            # first 256 queries see it as future (e*V)
                            nc.tensor.matmul(
                                po[h][:, 0:256],
                                evab[h][:, kt, 0:65],
                                rhs[:, 0:256],
                                start=False, stop=False)
                            # stop only on the final matmul (the whole
                            # [65,512] tile is one 2KB psum zero region)
                            nc.tensor.matmul(
                                po[h][:, 256:512],
                                vab[h][:, kt, 0:65],
                                rhs[:, 256:512],
                                start=False, stop=last)
                        else:
                            vv = evab[h] if fk > 2 * s + 1 else vab[h]
                            nc.tensor.matmul(
                                po[h][:, :],
                                vv[:, kt, 0:65],
                                rhs,
                                start=first, stop=last)

                    # previous stripe's outproj, spread one (tt, half) per
                    # ktile so the exp pipeline never starves
                    if pending is not None and 4 <= kt < 12:
                        emit_outproj(pending[0], pending[1],
                                     (kt - 4) // 2, (kt - 4) % 2)
                        if kt == 11:
                            pending = None
                # free po quickly: unnormalized o + denominators out
                zr_s = emit_norm(s, po)
                pending = (s, zr_s)
            for tt in range(4):
                for half in range(2):
                    emit_outproj(pending[0], pending[1], tt, half)

            if debug:
                for nm, t in (("dbg_oTn0", oTn0), ("dbg_oTn1", oTn1)):
                    stg = sba.tile([64, n_tok], f32, tag=f"stg_{nm}")
                    nc.scalar.copy(stg, t)
                    nc.sync.dma_start(dbg[nm], stg)

            sba_ctx.close()
            ps2_ctx.close()
            ctx.close()

    nc.compile()
    return nc


def shard_inputs(x, Wqkv, bqkv, gq, gk, Wout, n_tok):
    """Build the 8 per-core input maps (head-parallel sharding)."""
    D = D_MODEL
    in_maps = []
    for c in range(N_CORES):
        cs = slice(128 * c, 128 * (c + 1))
        wq = Wqkv[:, cs]
        wk = Wqkv[:, D + 128 * c:D + 128 * (c + 1)]
        wv = Wqkv[:, 2 * D + 128 * c:2 * D + 128 * (c + 1)]
        wqkv_s = np.ascontiguousarray(np.concatenate([wq, wk, wv], axis=1),
                                      dtype=np.float32)
        bq = bqkv[cs]
        bk = bqkv[D + 128 * c:D + 128 * (c + 1)]
        bv = bqkv[2 * D + 128 * c:2 * D + 128 * (c + 1)]
        bqkv_s = np.ascontiguousarray(np.concatenate([bq, bk, bv]),
                                      dtype=np.float32)
        gv = np.stack([np.concatenate([gq, gq]),
                       np.concatenate([gk, gk])], axis=1).astype(np.float32)
        wout_s = np.ascontiguousarray(Wout[cs, :], dtype=np.float32)
        in_maps.append({
            "x": np.ascontiguousarray(x[:n_tok], dtype=np.float32),
            "wqkv": wqkv_s,
            "bqkv": bqkv_s,
            "gv": np.ascontiguousarray(gv),
            "wout": wout_s,
        })
    return in_maps


_PROGRAM_CACHE = {}


def _get_program(n_tok):
    if n_tok not in _PROGRAM_CACHE:
        _PROGRAM_CACHE[n_tok] = build_program(n_tok)
    return _PROGRAM_CACHE[n_tok]


def run_sharded(inputs, trace=False, tmpdir=None):
    """Run the SPMD kernel; returns (full_output [1,N,D], BassKernelResults)."""
    from concourse.bass_utils import run_bass_kernel_spmd

    x = np.asarray(inputs["x"], dtype=np.float32)
    Wqkv = np.asarray(inputs["Wqkv"], dtype=np.float32)
    bqkv = np.asarray(inputs["bqkv"], dtype=np.float32)
    Wout = np.asarray(inputs["Wout"], dtype=np.float32)
    bout = np.asarray(inputs["bout"], dtype=np.float32)
    gq = np.asarray(inputs["gq"], dtype=np.float32)
    gk = np.asarray(inputs["gk"], dtype=np.float32)
    tpf = int(np.asarray(inputs["tokens_per_frame"]))
    assert tpf == TPF, f"kernel hardcodes tokens_per_frame={TPF}, got {tpf}"

    B, N, D = x.shape
    assert B == 1 and D == D_MODEL
    x2 = x[0]

    nc = _get_program(N)
    in_maps = shard_inputs(x2, Wqkv, bqkv, gq, gk, Wout, N)
    res = run_bass_kernel_spmd(nc, in_maps, list(range(N_CORES)),
                               trace=trace, tmpdir=tmpdir)
    acc = res.results[0]["out"].astype(np.float32)
    for c in range(1, N_CORES):
        acc = acc + res.results[c]["out"]
    if np.any(bout):
        acc = acc + bout[None, :]
    return acc[None], res


def kernel(**inputs):
    out, _ = run_sharded(inputs)
    return out
